# revision 14
# baseline (speedup 1.0000x reference)
"""LoRA basis-bank kernel for 8 TRN2 NeuronCores.

Math (per batch b):
    A_mixed  = sum_k alpha[b,k] * A_bank[k]        # [R, DIN]
    B_mixedT = sum_k alpha[b,k] * B_bank[k].T      # [R, DOUT]
    z        = h[b] @ A_mixed.T                    # [S, R]
    delta[b] = z @ B_mixedT                        # [S, DOUT]

Sharding: data-parallel over batch, 1 batch per core; banks replicated.

Host-side layout prep (no arithmetic): h shard uploaded transposed
(hT[i, s]) bf16; B_bank uploaded as [K, R, DOUT]; alpha expanded into a
[K*R, R] block-diagonal placement matrix; banks bf16.

v5 device dataflow. The TRN2 PE clock ramps with CONTINUOUS execution
(full 2.4 GHz only after ~3us without an idle gap, dropping to 1.2 GHz
after any bubble), so the kernel is built to keep the PE streaming:
  - hT as 16 resident [128, 2048] tiles (4KB DMA descriptors), split
    across both HWDGE queues; mm1 consumes chunks in DMA ARRIVAL order.
  - A_mixT mixing is JIT-interleaved into mm1 chunk by chunk (software
    pipelined one chunk ahead, rotating PSUM scratch) so per-chunk PE
    work (~1.3us) matches the per-tile DMA supply rate.
  - zT packed two s-chunks per PSUM bank at PE column positions 0/64;
    B_mixedT replicated at partition groups 0/64 for row-banded mm2.
  - mm2 emits small keep-warm filler matmuls (into a dead scratch bank)
    between output units so cast-gated gaps don't reset the PE clock.
  - Output casts per tile split [128,1024] DVE + [128,1024] ACT;
    delta stores ride the SP queue behind its loads.
"""

import ml_dtypes
import numpy as np

import concourse.bacc as bacc
import concourse.bass as bass
import concourse.mybir as mybir
import concourse.tile as tile
from concourse.bass_utils import run_bass_kernel_spmd

B, S, K, R, DIN, DOUT = 8, 2048, 16, 16, 2048, 2048
KR = K * R  # 256
F32 = mybir.dt.float32
BF16 = mybir.dt.bfloat16

_cache = {}

# mm1 chunk processing order ~= DMA arrival order (SP delivers c0-9,
# ACT delivers c10-15 after the banks, both queues concurrent).
_ARRIVAL = [0, 1, 2, 3, 4, 10, 5, 11, 6, 12, 7, 13, 8, 14, 9, 15]
# mm2 s-tile order: alternate sc parity -> adjacent matmuls use PE row
# bands 0/64.
_ST_ORDER = [0, 4, 1, 5, 2, 6, 3, 7, 8, 12, 9, 13, 10, 14, 11, 15]


def _build_nc():
    nc = bacc.Bacc("TRN2", target_bir_lowering=False,
                   enable_partition_id=False)

    ht_d = nc.dram_tensor("hbT", [DIN, S], BF16, kind="ExternalInput")
    mix_d = nc.dram_tensor("mix", [KR, R], BF16, kind="ExternalInput")
    a_d = nc.dram_tensor("a_flat", [KR, DIN], BF16, kind="ExternalInput")
    bt_d = nc.dram_tensor("bt_flat", [KR, DOUT], BF16, kind="ExternalInput")
    out_d = nc.dram_tensor("delta", [S, DOUT], BF16, kind="ExternalOutput")

    NCH = DIN // 128  # 16 chunks along DIN
    NSC = S // 512    # 4 s-chunks
    N_SP_LOADS = 10
    with tile.TileContext(nc) as tc:
        with (
            tc.tile_pool(name="hT", bufs=1) as hTp,
            tc.tile_pool(name="banks", bufs=1) as bankp,
            tc.tile_pool(name="mixed", bufs=1) as mixp,
            tc.tile_pool(name="dout", bufs=3) as dp,
        ):
            # ---- hT loads, first batch on SP ----
            hTs = []
            for c in range(NCH):
                hT = hTp.tile([128, S], BF16, tag=f"hT{c}")
                if c < N_SP_LOADS:
                    nc.sync.dma_start(hT[:], ht_d[c * 128:(c + 1) * 128, :])
                hTs.append(hT)

            # ---- banks on ACT, then its share of hT tiles ----
            m_sb, a_sb, b_sb = [], [], []
            for half in range(2):
                m_t = bankp.tile([128, R], BF16, tag=f"m{half}")
                nc.scalar.dma_start(m_t[:], mix_d[half * 128:(half + 1) * 128, :])
                m_sb.append(m_t)
            for half in range(2):
                a_t = bankp.tile([128, DIN], BF16, tag=f"a{half}")
                nc.scalar.dma_start(a_t[:], a_d[half * 128:(half + 1) * 128, :])
                a_sb.append(a_t)
            for half in range(2):
                b_t = bankp.tile([128, DOUT], BF16, tag=f"b{half}")
                nc.scalar.dma_start(b_t[:], bt_d[half * 128:(half + 1) * 128, :])
                b_sb.append(b_t)
            for c in range(N_SP_LOADS, NCH):
                nc.scalar.dma_start(hTs[c][:], ht_d[c * 128:(c + 1) * 128, :])

            bmix2 = mixp.tile([128, DOUT], BF16, tag="bmix2")
            zt2 = [mixp.tile([128, 512], BF16, tag=f"zt2_{i}", name=f"zt2_{i}")
                   for i in range(2)]
            amixT = [mixp.tile([128, R], BF16, tag=f"amixT{c}",
                               name=f"amixT{c}") for c in range(NCH)]

            with tc.tile_pool(name="ps1", bufs=1, space="PSUM") as ps1p:
                zt_ps = [ps1p.tile([128, 512], F32, tag=f"ztall{i}",
                                   name=f"zt_ps{i}") for i in range(2)]

                def amix_chunk(c, slot):
                    csl = slice(c * 128, (c + 1) * 128)
                    pat = ps1p.tile([128, R], F32, tag=f"pat{slot}",
                                    name=f"pat{slot}")
                    nc.tensor.matmul(pat[:], a_sb[0][:, csl], m_sb[0][:],
                                     start=True, stop=False)
                    nc.tensor.matmul(pat[:], a_sb[1][:, csl], m_sb[1][:],
                                     start=False, stop=True)
                    nc.vector.tensor_copy(amixT[c][:], pat[:])

                def mm1_chunk(c, first, last):
                    for sc in range(NSC):
                        po = 64 * (sc % 2)
                        nc.tensor.matmul(
                            zt_ps[sc // 2][po:po + R, :], amixT[c][:],
                            hTs[c][:, sc * 512:(sc + 1) * 512],
                            start=first, stop=last)

                def bmix_chunk(c4):
                    sl = slice(c4 * 512, (c4 + 1) * 512)
                    pmix = ps1p.tile([R, 512], F32, tag="pmix")
                    nc.tensor.matmul(pmix[:], m_sb[0][:], b_sb[0][:, sl],
                                     start=True, stop=False)
                    nc.tensor.matmul(pmix[:], m_sb[1][:], b_sb[1][:, sl],
                                     start=False, stop=True)
                    nc.vector.tensor_copy(bmix2[0:R, sl], pmix[:])

                # JIT mixing + mm1, software-pipelined one chunk ahead so
                # each amix PSUM->SBUF cast hides under the previous mm1.
                bmix_after = {7: 0, 9: 1, 11: 2, 13: 3}
                amix_chunk(_ARRIVAL[0], 0)
                for idx in range(NCH):
                    if idx + 1 < NCH:
                        amix_chunk(_ARRIVAL[idx + 1], (idx + 1) % 2)
                    mm1_chunk(_ARRIVAL[idx], first=(idx == 0),
                              last=(idx == NCH - 1))
                    if idx in bmix_after:
                        bmix_chunk(bmix_after[idx])
                nc.vector.tensor_copy(bmix2[64:64 + R, :], bmix2[0:R, :])
                for i in range(2):
                    nc.vector.tensor_copy(zt2[i][:], zt_ps[i][:])

            with (
                tc.tile_pool(name="psd", bufs=3, space="PSUM") as psdp,
                tc.tile_pool(name="psf", bufs=1, space="PSUM") as psfp,
            ):
                # Dead scratch bank for PE keep-warm fillers.
                fill = psfp.tile([128, 512], F32, tag="fill")

                def warm(n_cols):
                    nc.tensor.matmul(fill[0:R, 0:n_cols], amixT[0][:],
                                     hTs[0][:, 0:n_cols],
                                     start=True, stop=True,
                                     skip_group_check=True)

                # bridge the zt-cast boundary without a PE idle gap
                warm(512)
                warm(512)

                for ti, st in enumerate(_ST_ORDER):
                    sc = st // 4
                    po = 64 * (sc % 2)
                    row0 = st * 128
                    scol = slice((st % 4) * 128, (st % 4) * 128 + 128)
                    dsb = dp.tile([128, DOUT], BF16, tag="d")
                    for hp in range(2):
                        dps = psdp.tile([128, 1024], F32, tag="dps")
                        for q in range(2):
                            osl = slice((2 * hp + q) * 512,
                                        (2 * hp + q) * 512 + 512)
                            nc.tensor.matmul(
                                dps[:, q * 512:(q + 1) * 512],
                                zt2[sc // 2][po:po + R, scol],
                                bmix2[po:po + R, osl])
                        warm(128)
                        dcol = slice(hp * 1024, (hp + 1) * 1024)
                        if hp == 0:
                            nc.vector.tensor_copy(dsb[:, dcol], dps[:])
                        else:
                            nc.scalar.copy(dsb[:, dcol], dps[:])
                    nc.sync.dma_start(out_d[row0:row0 + 128, :], dsb[:])

    nc.compile()
    return nc


def _in_maps(h, alpha, A_bank, B_bank):
    a_flat = np.ascontiguousarray(
        A_bank.reshape(KR, DIN)).astype(ml_dtypes.bfloat16)
    bt_flat = np.ascontiguousarray(
        B_bank.transpose(0, 2, 1).reshape(KR, DOUT)).astype(ml_dtypes.bfloat16)
    eye = np.eye(R, dtype=np.float32)
    maps = []
    for b in range(B):
        mix = np.kron(alpha[b].astype(np.float32).reshape(K, 1),
                      eye).astype(ml_dtypes.bfloat16)
        hT = np.ascontiguousarray(
            np.asarray(h[b]).T).astype(ml_dtypes.bfloat16)
        maps.append({
            "hbT": hT,
            "mix": np.ascontiguousarray(mix),
            "a_flat": a_flat,
            "bt_flat": bt_flat,
        })
    return maps


def _run(inputs, trace=False):
    if "nc" not in _cache:
        _cache["nc"] = _build_nc()
    nc = _cache["nc"]
    maps = _in_maps(inputs["h"], inputs["alpha"], inputs["A_bank"],
                    inputs["B_bank"])
    res = run_bass_kernel_spmd(nc, maps, core_ids=list(range(B)), trace=trace)
    out = np.stack([res.results[b]["delta"] for b in range(B)], axis=0)
    return out.astype(np.float32), res


def kernel(**inputs):
    out, _ = _run(inputs, trace=False)
    return out


# revision 18
# speedup vs baseline: 1.0705x; 1.0705x over previous
"""LoRA basis-bank kernel for 8 TRN2 NeuronCores.

Math (per batch b):
    A_mixed  = sum_k alpha[b,k] * A_bank[k]        # [R, DIN]
    B_mixedT = sum_k alpha[b,k] * B_bank[k].T      # [R, DOUT]
    z        = h[b] @ A_mixed.T                    # [S, R]
    delta[b] = z @ B_mixedT                        # [S, DOUT]

Sharding: data-parallel over batch, 1 batch per core; banks replicated.

Host-side layout prep (no arithmetic): h shard uploaded transposed
(hT[i, s]) bf16; B_bank uploaded as [K, R, DOUT]; alpha expanded into a
[K*R, R] block-diagonal placement matrix; banks bf16.

v6 device dataflow. The TRN2 PE clock ramps with CONTINUOUS execution
(drops toward 1.2 GHz after idle bubbles), so the kernel keeps the PE
streaming where the data flow allows:
  - hT as 16 resident [128, 2048] tiles (4KB DMA descriptors), split
    across both HWDGE queues; mm1 consumes chunks in DMA ARRIVAL order.
  - A_mixT mixing is JIT-interleaved into mm1 chunk by chunk (software
    pipelined one chunk ahead, rotating PSUM scratch) so per-chunk PE
    work (~1.3us) matches the per-tile DMA supply rate.
  - zT packed two s-chunks per PSUM bank at PE column positions 0/64;
    B_mixedT replicated at partition groups 0/64 for row-banded mm2;
    zT casts interleaved into the final mm1 chunk to shrink the
    load->store phase boundary.
  - Output casts per tile split [128,1024] DVE + [128,1024] ACT;
    delta stores ride the SP queue behind its loads.
"""

import ml_dtypes
import numpy as np

import concourse.bacc as bacc
import concourse.bass as bass
import concourse.mybir as mybir
import concourse.tile as tile
from concourse.bass_utils import run_bass_kernel_spmd

B, S, K, R, DIN, DOUT = 8, 2048, 16, 16, 2048, 2048
KR = K * R  # 256
F32 = mybir.dt.float32
BF16 = mybir.dt.bfloat16

_cache = {}

# mm1 chunk processing order ~= DMA arrival order (SP delivers c0-9,
# ACT delivers c10-15 after the banks, both queues concurrent).
_ARRIVAL = [0, 1, 2, 3, 4, 10, 5, 11, 6, 12, 7, 13, 8, 14, 9, 15]
# mm2 s-tile order: alternate sc parity -> adjacent matmuls use PE row
# bands 0/64.
_ST_ORDER = [0, 4, 1, 5, 2, 6, 3, 7, 8, 12, 9, 13, 10, 14, 11, 15]


def _build_nc():
    nc = bacc.Bacc("TRN2", target_bir_lowering=False,
                   enable_partition_id=False)

    ht_d = nc.dram_tensor("hbT", [DIN, S], BF16, kind="ExternalInput")
    mix_d = nc.dram_tensor("mix", [KR, R], BF16, kind="ExternalInput")
    a_d = nc.dram_tensor("a_flat", [KR, DIN], BF16, kind="ExternalInput")
    bt_d = nc.dram_tensor("bt_flat", [KR, DOUT], BF16, kind="ExternalInput")
    out_d = nc.dram_tensor("delta", [S, DOUT], BF16, kind="ExternalOutput")

    NCH = DIN // 128  # 16 chunks along DIN
    NSC = S // 512    # 4 s-chunks
    N_SP_LOADS = 10
    with tile.TileContext(nc) as tc:
        with (
            tc.tile_pool(name="hT", bufs=1) as hTp,
            tc.tile_pool(name="banks", bufs=1) as bankp,
            tc.tile_pool(name="mixed", bufs=1) as mixp,
            tc.tile_pool(name="dout", bufs=3) as dp,
        ):
            # ---- hT loads, first batch on SP ----
            hTs = []
            for c in range(NCH):
                hT = hTp.tile([128, S], BF16, tag=f"hT{c}")
                if c < N_SP_LOADS:
                    nc.sync.dma_start(hT[:], ht_d[c * 128:(c + 1) * 128, :])
                hTs.append(hT)

            # ---- banks on ACT, then its share of hT tiles ----
            m_sb, a_sb, b_sb = [], [], []
            for half in range(2):
                m_t = bankp.tile([128, R], BF16, tag=f"m{half}")
                nc.scalar.dma_start(m_t[:], mix_d[half * 128:(half + 1) * 128, :])
                m_sb.append(m_t)
            for half in range(2):
                a_t = bankp.tile([128, DIN], BF16, tag=f"a{half}")
                nc.scalar.dma_start(a_t[:], a_d[half * 128:(half + 1) * 128, :])
                a_sb.append(a_t)
            for half in range(2):
                b_t = bankp.tile([128, DOUT], BF16, tag=f"b{half}")
                nc.scalar.dma_start(b_t[:], bt_d[half * 128:(half + 1) * 128, :])
                b_sb.append(b_t)
            for c in range(N_SP_LOADS, NCH):
                nc.scalar.dma_start(hTs[c][:], ht_d[c * 128:(c + 1) * 128, :])

            bmix2 = mixp.tile([128, DOUT], BF16, tag="bmix2")
            zt2 = [mixp.tile([128, 512], BF16, tag=f"zt2_{i}", name=f"zt2_{i}")
                   for i in range(2)]
            amixT = [mixp.tile([128, R], BF16, tag=f"amixT{c}",
                               name=f"amixT{c}") for c in range(NCH)]

            with tc.tile_pool(name="ps1", bufs=1, space="PSUM") as ps1p:
                zt_ps = [ps1p.tile([128, 512], F32, tag=f"ztall{i}",
                                   name=f"zt_ps{i}") for i in range(2)]

                def amix_chunk(c, slot):
                    csl = slice(c * 128, (c + 1) * 128)
                    pat = ps1p.tile([128, R], F32, tag=f"pat{slot}",
                                    name=f"pat{slot}")
                    nc.tensor.matmul(pat[:], a_sb[0][:, csl], m_sb[0][:],
                                     start=True, stop=False)
                    nc.tensor.matmul(pat[:], a_sb[1][:, csl], m_sb[1][:],
                                     start=False, stop=True)
                    nc.vector.tensor_copy(amixT[c][:], pat[:])

                def mm1_chunk(c, first, last):
                    for sc in range(NSC):
                        po = 64 * (sc % 2)
                        nc.tensor.matmul(
                            zt_ps[sc // 2][po:po + R, :], amixT[c][:],
                            hTs[c][:, sc * 512:(sc + 1) * 512],
                            start=first, stop=last)

                def bmix_chunk(c4):
                    sl = slice(c4 * 512, (c4 + 1) * 512)
                    pmix = ps1p.tile([R, 512], F32, tag="pmix")
                    nc.tensor.matmul(pmix[:], m_sb[0][:], b_sb[0][:, sl],
                                     start=True, stop=False)
                    nc.tensor.matmul(pmix[:], m_sb[1][:], b_sb[1][:, sl],
                                     start=False, stop=True)
                    nc.vector.tensor_copy(bmix2[0:R, sl], pmix[:])

                # JIT mixing + mm1, software-pipelined one chunk ahead so
                # each amix PSUM->SBUF cast hides under the previous mm1.
                bmix_after = {6: 0, 8: 1, 10: 2, 12: 3}
                amix_chunk(_ARRIVAL[0], 0)
                for idx in range(NCH - 1):
                    amix_chunk(_ARRIVAL[idx + 1], (idx + 1) % 2)
                    mm1_chunk(_ARRIVAL[idx], first=(idx == 0), last=False)
                    if idx in bmix_after:
                        bmix_chunk(bmix_after[idx])
                    if idx == 13:
                        nc.vector.tensor_copy(bmix2[64:64 + R, :],
                                              bmix2[0:R, :])
                # last chunk: interleave the zT casts between its s-chunk
                # pairs so they hide under the final matmuls
                c_last = _ARRIVAL[NCH - 1]
                for sc in (0, 1):
                    po = 64 * (sc % 2)
                    nc.tensor.matmul(
                        zt_ps[0][po:po + R, :], amixT[c_last][:],
                        hTs[c_last][:, sc * 512:(sc + 1) * 512],
                        start=False, stop=True)
                nc.vector.tensor_copy(zt2[0][:], zt_ps[0][:])
                for sc in (2, 3):
                    po = 64 * (sc % 2)
                    nc.tensor.matmul(
                        zt_ps[1][po:po + R, :], amixT[c_last][:],
                        hTs[c_last][:, sc * 512:(sc + 1) * 512],
                        start=False, stop=True)
                nc.vector.tensor_copy(zt2[1][:], zt_ps[1][:])

            with tc.tile_pool(name="psd", bufs=4, space="PSUM") as psdp:
                for ti, st in enumerate(_ST_ORDER):
                    sc = st // 4
                    po = 64 * (sc % 2)
                    row0 = st * 128
                    scol = slice((st % 4) * 128, (st % 4) * 128 + 128)
                    dsb = dp.tile([128, DOUT], BF16, tag="d")
                    for hp in range(2):
                        dps = psdp.tile([128, 1024], F32, tag="dps")
                        for q in range(2):
                            osl = slice((2 * hp + q) * 512,
                                        (2 * hp + q) * 512 + 512)
                            nc.tensor.matmul(
                                dps[:, q * 512:(q + 1) * 512],
                                zt2[sc // 2][po:po + R, scol],
                                bmix2[po:po + R, osl])
                        dcol = slice(hp * 1024, (hp + 1) * 1024)
                        if hp == 0:
                            nc.vector.tensor_copy(dsb[:, dcol], dps[:])
                        else:
                            nc.scalar.copy(dsb[:, dcol], dps[:])
                    nc.sync.dma_start(out_d[row0:row0 + 128, :], dsb[:])

    nc.compile()
    return nc


def _in_maps(h, alpha, A_bank, B_bank):
    a_flat = np.ascontiguousarray(
        A_bank.reshape(KR, DIN)).astype(ml_dtypes.bfloat16)
    bt_flat = np.ascontiguousarray(
        B_bank.transpose(0, 2, 1).reshape(KR, DOUT)).astype(ml_dtypes.bfloat16)
    eye = np.eye(R, dtype=np.float32)
    maps = []
    for b in range(B):
        mix = np.kron(alpha[b].astype(np.float32).reshape(K, 1),
                      eye).astype(ml_dtypes.bfloat16)
        hT = np.ascontiguousarray(
            np.asarray(h[b]).T).astype(ml_dtypes.bfloat16)
        maps.append({
            "hbT": hT,
            "mix": np.ascontiguousarray(mix),
            "a_flat": a_flat,
            "bt_flat": bt_flat,
        })
    return maps


def _run(inputs, trace=False):
    if "nc" not in _cache:
        _cache["nc"] = _build_nc()
    nc = _cache["nc"]
    maps = _in_maps(inputs["h"], inputs["alpha"], inputs["A_bank"],
                    inputs["B_bank"])
    res = run_bass_kernel_spmd(nc, maps, core_ids=list(range(B)), trace=trace)
    out = np.stack([res.results[b]["delta"] for b in range(B)], axis=0)
    return out.astype(np.float32), res


def kernel(**inputs):
    out, _ = _run(inputs, trace=False)
    return out


# revision 19
# speedup vs baseline: 1.0895x; 1.0177x over previous
"""LoRA basis-bank kernel for 8 TRN2 NeuronCores.

Math (per batch b):
    A_mixed  = sum_k alpha[b,k] * A_bank[k]        # [R, DIN]
    B_mixedT = sum_k alpha[b,k] * B_bank[k].T      # [R, DOUT]
    z        = h[b] @ A_mixed.T                    # [S, R]
    delta[b] = z @ B_mixedT                        # [S, DOUT]

Sharding: data-parallel over batch, 1 batch per core; banks replicated.

Host-side layout prep (no arithmetic): h shard uploaded transposed
(hT[i, s]) bf16; B_bank uploaded as [K, R, DOUT]; alpha expanded into a
[K*R, R] block-diagonal placement matrix; banks bf16.

v7 device dataflow:
  - hT as 16 resident [128, 2048] tiles (4KB DMA descriptors), split
    SP 11 / ACT 5 behind the banks; mm1 consumes chunks in DMA ARRIVAL
    order, with A_mixT mixing JIT-interleaved one chunk ahead.
  - zT packed two s-chunks per PSUM bank at PE column positions 0/64;
    B_mixedT replicated at partition groups 0/64.
  - PSUM pools are scoped so the mixing scratch closes before the last
    mm1 chunk and the mm2 pool opens BEFORE it — no pool-drain barrier
    at the load->store boundary; zT casts interleave into the final
    mm1 matmuls.
  - mm2 processes s-tile PAIRS with alternating PE row bands (0/64) so
    adjacent matmuls touch disjoint PE quadrants and PSUM banks.
  - Output casts per [128,1024] unit alternate DVE/ACT; delta stores
    ride the SP queue behind its loads.
"""

import ml_dtypes
import numpy as np

import concourse.bacc as bacc
import concourse.bass as bass
import concourse.mybir as mybir
import concourse.tile as tile
from concourse.bass_utils import run_bass_kernel_spmd

B, S, K, R, DIN, DOUT = 8, 2048, 16, 16, 2048, 2048
KR = K * R  # 256
F32 = mybir.dt.float32
BF16 = mybir.dt.bfloat16

_cache = {}

# mm1 chunk processing order ~= DMA arrival order (SP delivers c0-10,
# ACT delivers c11-15 after the banks, both queues concurrent).
_ARRIVAL = [0, 1, 2, 3, 4, 11, 5, 12, 6, 13, 7, 14, 8, 15, 9, 10]
# mm2 s-tile pair order: (po=0 tile, po=64 tile) pairs.
_ST_PAIRS = [(0, 4), (1, 5), (2, 6), (3, 7),
             (8, 12), (9, 13), (10, 14), (11, 15)]


def _build_nc():
    nc = bacc.Bacc("TRN2", target_bir_lowering=False,
                   enable_partition_id=False)

    ht_d = nc.dram_tensor("hbT", [DIN, S], BF16, kind="ExternalInput")
    mix_d = nc.dram_tensor("mix", [KR, R], BF16, kind="ExternalInput")
    a_d = nc.dram_tensor("a_flat", [KR, DIN], BF16, kind="ExternalInput")
    bt_d = nc.dram_tensor("bt_flat", [KR, DOUT], BF16, kind="ExternalInput")
    out_d = nc.dram_tensor("delta", [S, DOUT], BF16, kind="ExternalOutput")

    NCH = DIN // 128  # 16 chunks along DIN
    NSC = S // 512    # 4 s-chunks
    N_SP_LOADS = 11
    with tile.TileContext(nc) as tc:
        with (
            tc.tile_pool(name="hT", bufs=1) as hTp,
            tc.tile_pool(name="banks", bufs=1) as bankp,
            tc.tile_pool(name="mixed", bufs=1) as mixp,
            tc.tile_pool(name="dout", bufs=3) as dp,
        ):
            # ---- hT loads, first batch on SP ----
            hTs = []
            for c in range(NCH):
                hT = hTp.tile([128, S], BF16, tag=f"hT{c}")
                if c < N_SP_LOADS:
                    nc.sync.dma_start(hT[:], ht_d[c * 128:(c + 1) * 128, :])
                hTs.append(hT)

            # ---- banks on ACT, then its share of hT tiles ----
            m_sb, a_sb, b_sb = [], [], []
            for half in range(2):
                m_t = bankp.tile([128, R], BF16, tag=f"m{half}")
                nc.scalar.dma_start(m_t[:], mix_d[half * 128:(half + 1) * 128, :])
                m_sb.append(m_t)
            for half in range(2):
                a_t = bankp.tile([128, DIN], BF16, tag=f"a{half}")
                nc.scalar.dma_start(a_t[:], a_d[half * 128:(half + 1) * 128, :])
                a_sb.append(a_t)
            for half in range(2):
                b_t = bankp.tile([128, DOUT], BF16, tag=f"b{half}")
                nc.scalar.dma_start(b_t[:], bt_d[half * 128:(half + 1) * 128, :])
                b_sb.append(b_t)
            for c in range(N_SP_LOADS, NCH):
                nc.scalar.dma_start(hTs[c][:], ht_d[c * 128:(c + 1) * 128, :])

            bmix2 = mixp.tile([128, DOUT], BF16, tag="bmix2")
            zt2 = [mixp.tile([128, 512], BF16, tag=f"zt2_{i}", name=f"zt2_{i}")
                   for i in range(2)]
            amixT = [mixp.tile([128, R], BF16, tag=f"amixT{c}",
                               name=f"amixT{c}") for c in range(NCH)]

            with tc.tile_pool(name="psz", bufs=1, space="PSUM") as pszp:
                zt_ps = [pszp.tile([128, 512], F32, tag=f"ztall{i}",
                                   name=f"zt_ps{i}") for i in range(2)]

                def mm1_chunk(c, first):
                    for sc in range(NSC):
                        po = 64 * (sc % 2)
                        nc.tensor.matmul(
                            zt_ps[sc // 2][po:po + R, :], amixT[c][:],
                            hTs[c][:, sc * 512:(sc + 1) * 512],
                            start=first, stop=False,
                            skip_group_check=True)

                with tc.tile_pool(name="psm", bufs=1, space="PSUM") as psmp:
                    def amix_chunk(c, slot):
                        csl = slice(c * 128, (c + 1) * 128)
                        pat = psmp.tile([128, R], F32, tag=f"pat{slot}",
                                        name=f"pat{slot}")
                        nc.tensor.matmul(pat[:], a_sb[0][:, csl], m_sb[0][:],
                                         start=True, stop=False)
                        nc.tensor.matmul(pat[:], a_sb[1][:, csl], m_sb[1][:],
                                         start=False, stop=True)
                        nc.vector.tensor_copy(amixT[c][:], pat[:])

                    def bmix_chunk(c4):
                        sl = slice(c4 * 512, (c4 + 1) * 512)
                        pmix = psmp.tile([R, 512], F32, tag="pmix")
                        nc.tensor.matmul(pmix[:], m_sb[0][:], b_sb[0][:, sl],
                                         start=True, stop=False)
                        nc.tensor.matmul(pmix[:], m_sb[1][:], b_sb[1][:, sl],
                                         start=False, stop=True)
                        nc.vector.tensor_copy(bmix2[0:R, sl], pmix[:])

                    # JIT mixing + mm1, pipelined one chunk ahead; the
                    # last chunk is handled below, after the mixing
                    # scratch pool closes and the mm2 pool is open.
                    bmix_after = {6: 0, 8: 1, 10: 2, 12: 3}
                    amix_chunk(_ARRIVAL[0], 0)
                    for idx in range(NCH - 1):
                        amix_chunk(_ARRIVAL[idx + 1], (idx + 1) % 2)
                        mm1_chunk(_ARRIVAL[idx], first=(idx == 0))
                        if idx in bmix_after:
                            bmix_chunk(bmix_after[idx])
                        if idx == 12:
                            nc.vector.tensor_copy(bmix2[64:64 + R, :],
                                                  bmix2[0:R, :])

                with tc.tile_pool(name="psd", bufs=3, space="PSUM") as psdp:
                    # last chunk: zT casts interleaved between its
                    # s-chunk pairs; mm2 pool already open so the first
                    # delta matmuls follow with no pool-drain barrier
                    c_last = _ARRIVAL[NCH - 1]
                    for sc in (0, 1):
                        po = 64 * (sc % 2)
                        nc.tensor.matmul(
                            zt_ps[0][po:po + R, :], amixT[c_last][:],
                            hTs[c_last][:, sc * 512:(sc + 1) * 512],
                            start=False, stop=(sc == 1),
                            skip_group_check=True)
                    nc.vector.tensor_copy(zt2[0][:], zt_ps[0][:])
                    for sc in (2, 3):
                        po = 64 * (sc % 2)
                        nc.tensor.matmul(
                            zt_ps[1][po:po + R, :], amixT[c_last][:],
                            hTs[c_last][:, sc * 512:(sc + 1) * 512],
                            start=False, stop=(sc == 3),
                            skip_group_check=True)
                    nc.vector.tensor_copy(zt2[1][:], zt_ps[1][:])

                    # ---- mm2 over s-tile pairs with alternating PE row
                    # bands; casts alternate DVE/ACT; stores on SP ----
                    def unit(st, hp):
                        sc = st // 4
                        po = 64 * (sc % 2)
                        scol = slice((st % 4) * 128, (st % 4) * 128 + 128)
                        dps = psdp.tile([128, 1024], F32, tag="dps")
                        for q in range(2):
                            osl = slice((2 * hp + q) * 512,
                                        (2 * hp + q) * 512 + 512)
                            nc.tensor.matmul(
                                dps[:, q * 512:(q + 1) * 512],
                                zt2[sc // 2][po:po + R, scol],
                                bmix2[po:po + R, osl])
                        return dps

                    for stA, stB in _ST_PAIRS:
                        dsbA = dp.tile([128, DOUT], BF16, tag="d",
                                       name="dsbA")
                        dsbB = dp.tile([128, DOUT], BF16, tag="d",
                                       name="dsbB")
                        for hp in range(2):
                            dpsA = unit(stA, hp)
                            dpsB = unit(stB, hp)
                            dcol = slice(hp * 1024, (hp + 1) * 1024)
                            if hp == 0:
                                nc.vector.tensor_copy(dsbA[:, dcol], dpsA[:])
                                nc.scalar.copy(dsbB[:, dcol], dpsB[:])
                            else:
                                nc.scalar.copy(dsbA[:, dcol], dpsA[:])
                                nc.vector.tensor_copy(dsbB[:, dcol], dpsB[:])
                        nc.sync.dma_start(
                            out_d[stA * 128:stA * 128 + 128, :], dsbA[:])
                        nc.sync.dma_start(
                            out_d[stB * 128:stB * 128 + 128, :], dsbB[:])

    nc.compile()
    return nc


def _in_maps(h, alpha, A_bank, B_bank):
    a_flat = np.ascontiguousarray(
        A_bank.reshape(KR, DIN)).astype(ml_dtypes.bfloat16)
    bt_flat = np.ascontiguousarray(
        B_bank.transpose(0, 2, 1).reshape(KR, DOUT)).astype(ml_dtypes.bfloat16)
    eye = np.eye(R, dtype=np.float32)
    maps = []
    for b in range(B):
        mix = np.kron(alpha[b].astype(np.float32).reshape(K, 1),
                      eye).astype(ml_dtypes.bfloat16)
        hT = np.ascontiguousarray(
            np.asarray(h[b]).T).astype(ml_dtypes.bfloat16)
        maps.append({
            "hbT": hT,
            "mix": np.ascontiguousarray(mix),
            "a_flat": a_flat,
            "bt_flat": bt_flat,
        })
    return maps


def _run(inputs, trace=False):
    if "nc" not in _cache:
        _cache["nc"] = _build_nc()
    nc = _cache["nc"]
    maps = _in_maps(inputs["h"], inputs["alpha"], inputs["A_bank"],
                    inputs["B_bank"])
    res = run_bass_kernel_spmd(nc, maps, core_ids=list(range(B)), trace=trace)
    out = np.stack([res.results[b]["delta"] for b in range(B)], axis=0)
    return out.astype(np.float32), res


def kernel(**inputs):
    out, _ = _run(inputs, trace=False)
    return out
